# revision 51
# baseline (speedup 1.0000x reference)
"""Multi-head attention kernel for Trainium2, sharded over 8 NeuronCores.

Problem: B=4, S=2048, D=256, H=8 dense transformer attention block
(per-head K/V/Q Linear projections + dot-product attention + output Linear).

Sharding: core = (batch b, head-group g); core 2*b+g handles batch b and
heads [4g, 4g+4). Each core computes its heads' contribution to the final
output Linear (Wo rows h::H belong to head h); the host sums the two
partial outputs per batch and adds the (host-folded) bias.

Algebraic folds (host-side, exact up to fp32 rounding):
  - scores = (kWk+bk)(qWq'+bq')^T with Wq'=Wq/16, bq'=bq/16 expands to
      k M q^T + ku[m] + (per-query terms)
    where M = Wk Wq'^T and ku = k (Wk bq'). The per-query terms are
    constant along the softmax axis (keys) and cancel; ku becomes the Exp
    activation's per-partition bias. So the kernel needs no Q projection
    and no K/Q bias adds at all.
  - AV+output: w^T (v Wv + bv) Wo_h = w^T (v W2) + bv Wo_h with
    W2 = Wv Wo_h, so AV directly produces output-space values (transposed;
    host transposes back) and bo' = bo + sum_h bv[h] Wo_h is added on host.
  - k/v/q are transposed to [D, S] on host so no on-chip transposes occur.

On-chip per core (matmuls float32r / bf16, fp32 PSUM accumulate):
  tT[d',m] = M^T-contract of kT          (the only "projection" of k)
  V2[m,f]  = v @ W2
  sT[m,n]  = tT.T @ qT                   (keys m on partitions)
  expT     = exp(sT + ku[m])             (no max subtraction; scores O(1))
  sum[n]   = ones-matmul over partitions of a DVE-reduced chunk sum
  out[f,n]+= (V2.T-contract @ expT) * (1/sum)   (accumulated over heads)
"""

import numpy as np
from contextlib import ExitStack

import concourse.bacc as bacc
import concourse.bass as bass
import concourse.tile as tile
from concourse import mybir
from concourse.bass_utils import run_bass_kernel_spmd

B, S, D, H = 4, 2048, 256, 8
P = 128
DC = D // P            # 2 contraction/e-tile chunks
HPC = H // 2           # 4 heads per core
QB = 512               # query-block (n) width; also projection m-block
NQB = S // QB          # 4 query blocks
MT = S // P            # 16 key tiles
F32 = mybir.dt.float32
F32R = mybir.dt.float32r
BF16 = mybir.dt.bfloat16
EXP = mybir.ActivationFunctionType.Exp


def build_program(repeat=1, psa=5, psr=3, ebufs=2, kqvbufs=1, wbufs=2, tbufs=2, rbufs=2, sbufs=2):
    nc = bacc.Bacc(None, target_bir_lowering=False)

    ktd = nc.dram_tensor("kt", [D, S], F32R, kind="ExternalInput")
    vtd = nc.dram_tensor("vt", [D, S], F32R, kind="ExternalInput")
    qtd = nc.dram_tensor("qt", [D, S], F32R, kind="ExternalInput")
    wmd = nc.dram_tensor("wm", [HPC, D, D], F32R, kind="ExternalInput")
    w2d = nc.dram_tensor("w2", [HPC, D, D], F32R, kind="ExternalInput")
    kud = nc.dram_tensor("ku", [HPC, P, MT], F32, kind="ExternalInput")
    outd = nc.dram_tensor("out", [D, S], F32, kind="ExternalOutput")

    with ExitStack() as ctx:
        tc = ctx.enter_context(tile.TileContext(nc))
        const = ctx.enter_context(tc.tile_pool(name="const", bufs=1))
        wpool = ctx.enter_context(tc.tile_pool(name="w", bufs=wbufs))
        kqv = ctx.enter_context(tc.tile_pool(name="kqv", bufs=kqvbufs))
        epool = ctx.enter_context(tc.tile_pool(name="exp", bufs=ebufs))
        tpool = ctx.enter_context(tc.tile_pool(name="tree", bufs=tbufs))
        rcpool = ctx.enter_context(tc.tile_pool(name="recip", bufs=rbufs))
        scpool = ctx.enter_context(tc.tile_pool(name="scratch", bufs=sbufs))
        psA = ctx.enter_context(
            tc.tile_pool(name="psA", bufs=psa, space=bass.MemorySpace.PSUM))
        psR = ctx.enter_context(
            tc.tile_pool(name="psR", bufs=psr, space=bass.MemorySpace.PSUM))

        ones_t = const.tile([P, P], BF16)
        nc.vector.memset(ones_t[:], 1.0)

        def load_weights_m(h):
            wm_sb = wpool.tile([P, DC * D], F32R, tag="wm")
            ku_sb = wpool.tile([P, MT], F32, tag="ku")
            for dc in range(DC):
                nc.sync.dma_start(wm_sb[:, dc * D:(dc + 1) * D],
                                  wmd[h, dc * P:(dc + 1) * P, :])
            nc.gpsimd.dma_start(ku_sb[:], kud[h])
            return wm_sb, ku_sb

        def load_weights_2(h):
            w2_sb = wpool.tile([P, DC * D], F32R, tag="w2")
            for dc in range(DC):
                nc.gpsimd.dma_start(w2_sb[:, dc * D:(dc + 1) * D],
                                    w2d[h, dc * P:(dc + 1) * P, :])
            return (w2_sb,)

        for _rep in range(repeat):
            _build_iteration(nc, const, wpool, kqv, epool, tpool, rcpool,
                             scpool, psA, psR, ones_t,
                             load_weights_m, load_weights_2,
                             ktd, vtd, qtd, outd)

    nc.compile()
    return nc


def _build_iteration(nc, const, wpool, kqv, epool, tpool, rcpool, scpool,
                     psA, psR, ones_t, load_weights_m, load_weights_2,
                     ktd, vtd, qtd, outd):
    w_m0 = load_weights_m(0)

    kT = const.tile([P, DC * S], F32R)
    vT = const.tile([P, DC * S], F32R)
    qT = const.tile([P, DC * S], F32R)
    # chunked loads, k/q on separate queues, interleaved in the order the
    # first projections consume them (dc-pairs per 512-wide m-range)
    for pc in range(NQB):
        for dc in range(DC):
            sl = slice(dc * S + pc * QB, dc * S + (pc + 1) * QB)
            dsl = slice(pc * QB, (pc + 1) * QB)
            nc.sync.dma_start(kT[:, sl], ktd[dc * P:(dc + 1) * P, dsl])
            nc.gpsimd.dma_start(qT[:, sl], qtd[dc * P:(dc + 1) * P, dsl])
    w_next = w_m0 + load_weights_2(0)
    HS = S // 2
    for half in range(2):
        for dc in range(DC):
            sl = slice(dc * S + half * HS, dc * S + (half + 1) * HS)
            dsl = slice(half * HS, (half + 1) * HS)
            nc.scalar.dma_start(vT[:, sl], vtd[dc * P:(dc + 1) * P, dsl])

    # out_acc[p, ft*S + n] accumulates out^T[f, n] over heads
    out_acc = const.tile([P, DC * S], F32)

    def tail(h, nb, expT, pair, racc=None):
        # softmax denominator: reduce the 16 key-chunks pairwise on
        # DVE, then a ones-matmul sums over partitions (broadcast).
        if racc is None:
            half_w = MT * QB // 2
            tmp = tpool.tile([P, half_w], BF16, tag="tree")
            nc.vector.tensor_add(tmp[:], expT[:, :half_w], expT[:, half_w:])
            w = half_w // 2
            while w >= QB:
                nc.vector.tensor_add(tmp[:, :w], tmp[:, :w], tmp[:, w:2 * w])
                w //= 2
            racc = tmp
        ps_sum = psA.tile([P, QB], F32, tag="psA")
        nc.tensor.matmul(ps_sum[:], ones_t[:], racc[:, :QB],
                         start=True, stop=True)
        recip = rcpool.tile([P, QB], F32, tag="recip")
        nc.vector.reciprocal(recip[:], ps_sum[:])
        final = (h == HPC - 1 and nb == NQB - 1)
        for et in range(DC):
            osl = slice(et * S + nb * QB, et * S + (nb + 1) * QB)
            # final block: the second accumulate runs on the idle GpSimd
            # (SBUF-only; GpSimd cannot read the PSUM mul operands)
            ae = nc.gpsimd if (final and et == 1) else nc.vector
            if h == 0:
                nc.vector.tensor_mul(out_acc[:, osl], pair[et][:], recip[:])
            else:
                sc = scpool.tile([P, QB], F32, tag="sc")
                nc.vector.tensor_mul(sc[:], pair[et][:], recip[:])
                ae.tensor_add(out_acc[:, osl], out_acc[:, osl], sc[:])
            if h == HPC - 1:
                eng = nc.sync if et == 0 else nc.gpsimd
                eng.dma_start(
                    outd[et * P:(et + 1) * P, nb * QB:(nb + 1) * QB],
                    out_acc[:, osl])

    pending = None          # (h, nb, expT, pair) awaiting tail
    for h in range(HPC):
        wm_sb, ku_sb, w2_sb = w_next
        if h + 1 < HPC:
            w_next = load_weights_m(h + 1) + load_weights_2(h + 1)

        tT_h = kqv.tile([P, DC * S], F32R, tag="tT")
        V2_h = kqv.tile([P, MT * D], BF16, tag="V2")

        # t-projection: psum[d'-tile, m-block] = sum_dc M[dc,et].T @ kT[dc,mb]
        # (mb-major so the first score tiles' lhsT evacuates first; the
        # evacuation burst alternates DVE/ACT to halve its critical path)
        for mb in range(NQB):
            for et in range(DC):
                msl = slice(et * S + mb * QB, et * S + (mb + 1) * QB)
                ps = psA.tile([P, QB], F32, tag="psA")
                for dc in range(DC):
                    nc.tensor.matmul(
                        ps[:],
                        wm_sb[:, dc * D + et * P:dc * D + (et + 1) * P],
                        kT[:, dc * S + mb * QB:dc * S + (mb + 1) * QB],
                        start=(dc == 0), stop=(dc == DC - 1))
                if et == 0:
                    nc.vector.tensor_copy(tT_h[:, msl], ps[:])
                else:
                    nc.scalar.activation(tT_h[:, msl], ps[:],
                                         mybir.ActivationFunctionType.Copy)

        # V2 projection: psum[m-tile, f] = sum_dc vT[dc,mt].T @ W2[dc]
        # (two m-tiles share one PSUM bank -> one evacuation copy each)
        for mp in range(MT // 2):
            ps = psA.tile([P, QB], F32, tag="psA")
            for half in range(2):
                mt = 2 * mp + half
                for dc in range(DC):
                    nc.tensor.matmul(
                        ps[:, half * D:(half + 1) * D],
                        vT[:, dc * S + mt * P:dc * S + (mt + 1) * P],
                        w2_sb[:, dc * D:(dc + 1) * D],
                        start=(dc == 0), stop=(dc == DC - 1))
            if mp % 2 == 0:
                nc.vector.tensor_copy(V2_h[:, 2 * mp * D:2 * (mp + 1) * D],
                                      ps[:])
            else:
                nc.scalar.activation(V2_h[:, 2 * mp * D:2 * (mp + 1) * D],
                                     ps[:],
                                     mybir.ActivationFunctionType.Copy)

        # Emission is software-pipelined S(nb) A(nb) S(nb+1) T(nb) ... so
        # the scheduler always has next-block scores ready for the PE
        # while block nb's tree/sum/normalize chain runs on DVE.
        def scores_exp(nb, running=False):
            expT = epool.tile([P, MT * QB], BF16, tag="exp")
            racc = None
            for mt in range(MT):
                ps = psA.tile([P, QB], F32, tag="psA")
                for ec in range(DC):
                    nc.tensor.matmul(
                        ps[:],
                        tT_h[:, ec * S + mt * P:ec * S + (mt + 1) * P],
                        qT[:, ec * S + nb * QB:ec * S + (nb + 1) * QB],
                        start=(ec == 0), stop=(ec == DC - 1))
                # exp(s + ku[m]) -- the folded K-side bias rides in the
                # activation's per-partition bias operand for free
                nc.scalar.activation(expT[:, mt * QB:(mt + 1) * QB], ps[:],
                                     EXP, bias=ku_sb[:, mt:mt + 1])
                if running and mt == 1:
                    racc = tpool.tile([P, QB], BF16, tag="racc")
                    nc.vector.tensor_add(racc[:], expT[:, :QB],
                                         expT[:, QB:2 * QB])
                elif running and mt > 1:
                    nc.vector.tensor_add(
                        racc[:], racc[:],
                        expT[:, mt * QB:(mt + 1) * QB])
            return expT, racc

        def av_one(expT, et):
            # out-space AV: psum[f-tile, n] = sum_mt V2[mt, et].T @ expT[mt]
            ps_rep = psR.tile([P, QB], F32, tag="psR")
            for mt in range(MT):
                nc.tensor.matmul(
                    ps_rep[:],
                    V2_h[:, mt * D + et * P:mt * D + (et + 1) * P],
                    expT[:, mt * QB:(mt + 1) * QB],
                    start=(mt == 0), stop=(mt == MT - 1))
            return ps_rep

        def av(expT):
            return [av_one(expT, et) for et in range(DC)]

        for nb in range(NQB):
            last = (h == HPC - 1 and nb == NQB - 1)
            expT, racc = scores_exp(nb, running=last)
            if pending is not None:
                tail(*pending)
                pending = None
            if not last:
                pair = av(expT)
                pending = (h, nb, expT, pair, racc)
                continue
            # Final block: emit the softmax-sum matmul and reciprocal
            # between the two AV groups (their racc dependency is ready
            # mid-AV), and normalize muls-before-adds, so the exposed
            # end-of-kernel chain is as short as possible.
            pair = [av_one(expT, 0)]
            ps_sum = psA.tile([P, QB], F32, tag="psA")
            nc.tensor.matmul(ps_sum[:], ones_t[:], racc[:, :QB],
                             start=True, stop=True)
            recip = rcpool.tile([P, QB], F32, tag="recip")
            nc.vector.reciprocal(recip[:], ps_sum[:])
            pair.append(av_one(expT, 1))
            scs = []
            for et in range(DC):
                sc = scpool.tile([P, QB], F32, tag="sc")
                nc.vector.tensor_mul(sc[:], pair[et][:], recip[:])
                scs.append(sc)
            for et in range(DC):
                osl = slice(et * S + nb * QB, et * S + (nb + 1) * QB)
                ae = nc.gpsimd if et == 1 else nc.vector
                ae.tensor_add(out_acc[:, osl], out_acc[:, osl], scs[et][:])
                eng = nc.sync if et == 0 else nc.gpsimd
                eng.dma_start(
                    outd[et * P:(et + 1) * P, nb * QB:(nb + 1) * QB],
                    out_acc[:, osl])



_progs = {}


def _get_prog(repeat=1):
    if repeat not in _progs:
        _progs[repeat] = build_program(repeat)
    return _progs[repeat]


def _prepare_in_maps(k, v, q, Wk, bk, Wv, bv, Wq, bq, Wo, bo):
    scale = np.float32(1.0 / 16.0)  # 1/sqrt(D), exact power of two
    in_maps = []
    for core in range(2 * B):
        b, g = core // 2, core % 2
        hs = list(range(g * HPC, (g + 1) * HPC))
        wm = np.stack([
            (Wk[h].astype(np.float64)
             @ (Wq[h].astype(np.float64) * scale).T).astype(np.float32)
            for h in hs])
        w2 = np.stack([
            (Wv[h].astype(np.float64)
             @ Wo[h::H].astype(np.float64)).astype(np.float32)
            for h in hs])
        ku = np.stack([
            (k[b].astype(np.float64)
             @ (Wk[h].astype(np.float64) @ (bq[h].astype(np.float64) * scale))
             ).astype(np.float32).reshape(MT, P).T
            for h in hs])
        in_maps.append({
            "kt": np.ascontiguousarray(k[b].T),
            "vt": np.ascontiguousarray(v[b].T),
            "qt": np.ascontiguousarray(q[b].T),
            "wm": np.ascontiguousarray(wm),
            "w2": np.ascontiguousarray(w2),
            "ku": np.ascontiguousarray(ku),
        })
    return in_maps


def _bo_prime(bv, Wo, bo):
    acc = bo.astype(np.float64).copy()
    for h in range(H):
        acc += bv[h].astype(np.float64) @ Wo[h::H].astype(np.float64)
    return acc.astype(np.float32)


def _run_spmd(in_maps, repeat=1, **kwargs):
    nc = _get_prog(repeat)
    return run_bass_kernel_spmd(nc, in_maps, core_ids=list(range(2 * B)),
                                **kwargs)


def kernel(k, v, q, Wk, bk, Wv, bv, Wq, bq, Wo, bo):
    arrs = [np.asarray(x, dtype=np.float32)
            for x in (k, v, q, Wk, bk, Wv, bv, Wq, bq, Wo, bo)]
    k, v, q, Wk, bk, Wv, bv, Wq, bq, Wo, bo = arrs
    in_maps = _prepare_in_maps(k, v, q, Wk, bk, Wv, bv, Wq, bq, Wo, bo)
    rr = _run_spmd(in_maps)
    bop = _bo_prime(bv, Wo, bo)
    out = np.empty((B, S, D), np.float32)
    for b in range(B):
        out[b] = (rr.results[2 * b]["out"].T + rr.results[2 * b + 1]["out"].T
                  + bop)
    return out


# revision 52
# speedup vs baseline: 1.1618x; 1.1618x over previous
"""Multi-head attention kernel for Trainium2, sharded over 8 NeuronCores.

Problem: B=4, S=2048, D=256, H=8 dense transformer attention block
(per-head K/V/Q Linear projections + dot-product attention + output Linear).

Sharding: core = (batch b, head-group g); core 2*b+g handles batch b and
heads [4g, 4g+4). Each core computes its heads' contribution to the final
output Linear (Wo rows h::H belong to head h); the host sums the two
partial outputs per batch and adds the (host-folded) bias.

Algebraic folds (host-side, exact up to fp32 rounding):
  - scores = (kWk+bk)(qWq'+bq')^T with Wq'=Wq/16, bq'=bq/16 expands to
      k M q^T + ku[m] + (per-query terms)
    where M = Wk Wq'^T and ku = k (Wk bq'). The per-query terms are
    constant along the softmax axis (keys) and cancel; ku becomes the Exp
    activation's per-partition bias. So the kernel needs no Q projection
    and no K/Q bias adds at all.
  - AV+output: w^T (v Wv + bv) Wo_h = w^T (v W2) + bv Wo_h with
    W2 = Wv Wo_h, so AV directly produces output-space values (transposed;
    host transposes back) and bo' = bo + sum_h bv[h] Wo_h is added on host.
  - k/v/q are transposed to [D, S] on host so no on-chip transposes occur.

On-chip per core (matmuls float32r / bf16, fp32 PSUM accumulate):
  tT[d',m] = M^T-contract of kT          (the only "projection" of k)
  V2[m,f]  = v @ W2
  sT[m,n]  = tT.T @ qT                   (keys m on partitions)
  expT     = exp(sT + ku[m])             (no max subtraction; scores O(1))
  sum[n]   = ones-matmul over partitions of a DVE-reduced chunk sum
  out[f,n]+= (V2.T-contract @ expT) * (1/sum)   (accumulated over heads)
"""

import numpy as np
from contextlib import ExitStack

import concourse.bacc as bacc
import concourse.bass as bass
import concourse.tile as tile
from concourse import mybir
from concourse.bass_utils import run_bass_kernel_spmd

B, S, D, H = 4, 2048, 256, 8
P = 128
DC = D // P            # 2 contraction/e-tile chunks
HPC = H // 2           # 4 heads per core
QB = 512               # query-block (n) width; also projection m-block
NQB = S // QB          # 4 query blocks
MT = S // P            # 16 key tiles
F32 = mybir.dt.float32
F32R = mybir.dt.float32r
BF16 = mybir.dt.bfloat16
EXP = mybir.ActivationFunctionType.Exp


def build_program(repeat=1, psa=5, psr=3, ebufs=2, kqvbufs=1, wbufs=2, tbufs=2, rbufs=2, sbufs=2):
    nc = bacc.Bacc(None, target_bir_lowering=False)

    ktd = nc.dram_tensor("kt", [D, S], F32R, kind="ExternalInput")
    vtd = nc.dram_tensor("vt", [D, S], F32R, kind="ExternalInput")
    qtd = nc.dram_tensor("qt", [D, S], F32R, kind="ExternalInput")
    wmd = nc.dram_tensor("wm", [HPC, D, D], F32R, kind="ExternalInput")
    w2d = nc.dram_tensor("w2", [HPC, D, D], F32R, kind="ExternalInput")
    kud = nc.dram_tensor("ku", [HPC, P, MT], F32, kind="ExternalInput")
    outd = nc.dram_tensor("out", [D, S], F32, kind="ExternalOutput")

    with ExitStack() as ctx:
        tc = ctx.enter_context(tile.TileContext(nc))
        const = ctx.enter_context(tc.tile_pool(name="const", bufs=1))
        wpool = ctx.enter_context(tc.tile_pool(name="w", bufs=wbufs))
        kqv = ctx.enter_context(tc.tile_pool(name="kqv", bufs=kqvbufs))
        epool = ctx.enter_context(tc.tile_pool(name="exp", bufs=ebufs))
        tpool = ctx.enter_context(tc.tile_pool(name="tree", bufs=tbufs))
        rcpool = ctx.enter_context(tc.tile_pool(name="recip", bufs=rbufs))
        scpool = ctx.enter_context(tc.tile_pool(name="scratch", bufs=sbufs))
        psA = ctx.enter_context(
            tc.tile_pool(name="psA", bufs=psa, space=bass.MemorySpace.PSUM))
        psR = ctx.enter_context(
            tc.tile_pool(name="psR", bufs=psr, space=bass.MemorySpace.PSUM))

        ones_t = const.tile([P, P], BF16)
        nc.vector.memset(ones_t[:], 1.0)

        def load_weights_m(h):
            wm_sb = wpool.tile([P, DC * D], F32R, tag="wm")
            ku_sb = wpool.tile([P, MT], F32, tag="ku")
            for dc in range(DC):
                nc.sync.dma_start(wm_sb[:, dc * D:(dc + 1) * D],
                                  wmd[h, dc * P:(dc + 1) * P, :])
            nc.gpsimd.dma_start(ku_sb[:], kud[h])
            return wm_sb, ku_sb

        def load_weights_2(h):
            w2_sb = wpool.tile([P, DC * D], F32R, tag="w2")
            for dc in range(DC):
                nc.gpsimd.dma_start(w2_sb[:, dc * D:(dc + 1) * D],
                                    w2d[h, dc * P:(dc + 1) * P, :])
            return (w2_sb,)

        for _rep in range(repeat):
            _build_iteration(nc, const, wpool, kqv, epool, tpool, rcpool,
                             scpool, psA, psR, ones_t,
                             load_weights_m, load_weights_2,
                             ktd, vtd, qtd, outd)

    nc.compile()
    return nc


def _build_iteration(nc, const, wpool, kqv, epool, tpool, rcpool, scpool,
                     psA, psR, ones_t, load_weights_m, load_weights_2,
                     ktd, vtd, qtd, outd):
    # Warm the PE during the initial input-DMA wait: tiny ones-matmuls
    # with no DMA dependencies keep the array busy through the cold
    # p-state / HAM window, so the first real projections run warm.
    ps_warm = psA.tile([P, QB], F32, tag="psA")
    for wi in range(20):
        nc.tensor.matmul(ps_warm[:, :P], ones_t[:], ones_t[:],
                         start=(wi == 0), stop=(wi == 19))

    w_m0 = load_weights_m(0)

    kT = const.tile([P, DC * S], F32R)
    vT = const.tile([P, DC * S], F32R)
    qT = const.tile([P, DC * S], F32R)
    # chunked loads, k/q on separate queues, interleaved in the order the
    # first projections consume them (dc-pairs per 512-wide m-range)
    for pc in range(NQB):
        for dc in range(DC):
            sl = slice(dc * S + pc * QB, dc * S + (pc + 1) * QB)
            dsl = slice(pc * QB, (pc + 1) * QB)
            nc.sync.dma_start(kT[:, sl], ktd[dc * P:(dc + 1) * P, dsl])
            nc.gpsimd.dma_start(qT[:, sl], qtd[dc * P:(dc + 1) * P, dsl])
    w_next = w_m0 + load_weights_2(0)
    HS = S // 2
    for half in range(2):
        for dc in range(DC):
            sl = slice(dc * S + half * HS, dc * S + (half + 1) * HS)
            dsl = slice(half * HS, (half + 1) * HS)
            nc.scalar.dma_start(vT[:, sl], vtd[dc * P:(dc + 1) * P, dsl])

    # out_acc[p, ft*S + n] accumulates out^T[f, n] over heads
    out_acc = const.tile([P, DC * S], F32)

    def tail(h, nb, expT, pair, racc=None):
        # softmax denominator: reduce the 16 key-chunks pairwise on
        # DVE, then a ones-matmul sums over partitions (broadcast).
        if racc is None:
            half_w = MT * QB // 2
            tmp = tpool.tile([P, half_w], BF16, tag="tree")
            nc.vector.tensor_add(tmp[:], expT[:, :half_w], expT[:, half_w:])
            w = half_w // 2
            while w >= QB:
                nc.vector.tensor_add(tmp[:, :w], tmp[:, :w], tmp[:, w:2 * w])
                w //= 2
            racc = tmp
        ps_sum = psA.tile([P, QB], F32, tag="psA")
        nc.tensor.matmul(ps_sum[:], ones_t[:], racc[:, :QB],
                         start=True, stop=True)
        recip = rcpool.tile([P, QB], F32, tag="recip")
        nc.vector.reciprocal(recip[:], ps_sum[:])
        final = (h == HPC - 1 and nb == NQB - 1)
        for et in range(DC):
            osl = slice(et * S + nb * QB, et * S + (nb + 1) * QB)
            # final block: the second accumulate runs on the idle GpSimd
            # (SBUF-only; GpSimd cannot read the PSUM mul operands)
            ae = nc.gpsimd if (final and et == 1) else nc.vector
            if h == 0:
                nc.vector.tensor_mul(out_acc[:, osl], pair[et][:], recip[:])
            else:
                sc = scpool.tile([P, QB], F32, tag="sc")
                nc.vector.tensor_mul(sc[:], pair[et][:], recip[:])
                ae.tensor_add(out_acc[:, osl], out_acc[:, osl], sc[:])
            if h == HPC - 1:
                eng = nc.sync if et == 0 else nc.gpsimd
                eng.dma_start(
                    outd[et * P:(et + 1) * P, nb * QB:(nb + 1) * QB],
                    out_acc[:, osl])

    pending = None          # (h, nb, expT, pair) awaiting tail
    for h in range(HPC):
        wm_sb, ku_sb, w2_sb = w_next
        if h + 1 < HPC:
            w_next = load_weights_m(h + 1) + load_weights_2(h + 1)

        tT_h = kqv.tile([P, DC * S], F32R, tag="tT")
        V2_h = kqv.tile([P, MT * D], BF16, tag="V2")

        # t-projection: psum[d'-tile, m-block] = sum_dc M[dc,et].T @ kT[dc,mb]
        # (mb-major so the first score tiles' lhsT evacuates first; the
        # evacuation burst alternates DVE/ACT to halve its critical path)
        for mb in range(NQB):
            for et in range(DC):
                msl = slice(et * S + mb * QB, et * S + (mb + 1) * QB)
                ps = psA.tile([P, QB], F32, tag="psA")
                for dc in range(DC):
                    nc.tensor.matmul(
                        ps[:],
                        wm_sb[:, dc * D + et * P:dc * D + (et + 1) * P],
                        kT[:, dc * S + mb * QB:dc * S + (mb + 1) * QB],
                        start=(dc == 0), stop=(dc == DC - 1))
                if et == 0:
                    nc.vector.tensor_copy(tT_h[:, msl], ps[:])
                else:
                    nc.scalar.activation(tT_h[:, msl], ps[:],
                                         mybir.ActivationFunctionType.Copy)

        # V2 projection: psum[m-tile, f] = sum_dc vT[dc,mt].T @ W2[dc]
        # (two m-tiles share one PSUM bank -> one evacuation copy each)
        for mp in range(MT // 2):
            ps = psA.tile([P, QB], F32, tag="psA")
            for half in range(2):
                mt = 2 * mp + half
                for dc in range(DC):
                    nc.tensor.matmul(
                        ps[:, half * D:(half + 1) * D],
                        vT[:, dc * S + mt * P:dc * S + (mt + 1) * P],
                        w2_sb[:, dc * D:(dc + 1) * D],
                        start=(dc == 0), stop=(dc == DC - 1))
            if mp % 2 == 0:
                nc.vector.tensor_copy(V2_h[:, 2 * mp * D:2 * (mp + 1) * D],
                                      ps[:])
            else:
                nc.scalar.activation(V2_h[:, 2 * mp * D:2 * (mp + 1) * D],
                                     ps[:],
                                     mybir.ActivationFunctionType.Copy)

        # Emission is software-pipelined S(nb) A(nb) S(nb+1) T(nb) ... so
        # the scheduler always has next-block scores ready for the PE
        # while block nb's tree/sum/normalize chain runs on DVE.
        def scores_exp(nb, running=False):
            expT = epool.tile([P, MT * QB], BF16, tag="exp")
            racc = None
            for mt in range(MT):
                ps = psA.tile([P, QB], F32, tag="psA")
                for ec in range(DC):
                    nc.tensor.matmul(
                        ps[:],
                        tT_h[:, ec * S + mt * P:ec * S + (mt + 1) * P],
                        qT[:, ec * S + nb * QB:ec * S + (nb + 1) * QB],
                        start=(ec == 0), stop=(ec == DC - 1))
                # exp(s + ku[m]) -- the folded K-side bias rides in the
                # activation's per-partition bias operand for free
                nc.scalar.activation(expT[:, mt * QB:(mt + 1) * QB], ps[:],
                                     EXP, bias=ku_sb[:, mt:mt + 1])
                if running and mt == 1:
                    racc = tpool.tile([P, QB], BF16, tag="racc")
                    nc.vector.tensor_add(racc[:], expT[:, :QB],
                                         expT[:, QB:2 * QB])
                elif running and mt > 1:
                    nc.vector.tensor_add(
                        racc[:], racc[:],
                        expT[:, mt * QB:(mt + 1) * QB])
            return expT, racc

        def av_one(expT, et):
            # out-space AV: psum[f-tile, n] = sum_mt V2[mt, et].T @ expT[mt]
            ps_rep = psR.tile([P, QB], F32, tag="psR")
            for mt in range(MT):
                nc.tensor.matmul(
                    ps_rep[:],
                    V2_h[:, mt * D + et * P:mt * D + (et + 1) * P],
                    expT[:, mt * QB:(mt + 1) * QB],
                    start=(mt == 0), stop=(mt == MT - 1))
            return ps_rep

        def av(expT):
            return [av_one(expT, et) for et in range(DC)]

        for nb in range(NQB):
            last = (h == HPC - 1 and nb == NQB - 1)
            expT, racc = scores_exp(nb, running=last)
            if pending is not None:
                tail(*pending)
                pending = None
            if not last:
                pair = av(expT)
                pending = (h, nb, expT, pair, racc)
                continue
            # Final block: emit the softmax-sum matmul and reciprocal
            # between the two AV groups (their racc dependency is ready
            # mid-AV), and normalize muls-before-adds, so the exposed
            # end-of-kernel chain is as short as possible.
            pair = [av_one(expT, 0)]
            ps_sum = psA.tile([P, QB], F32, tag="psA")
            nc.tensor.matmul(ps_sum[:], ones_t[:], racc[:, :QB],
                             start=True, stop=True)
            recip = rcpool.tile([P, QB], F32, tag="recip")
            nc.vector.reciprocal(recip[:], ps_sum[:])
            pair.append(av_one(expT, 1))
            scs = []
            for et in range(DC):
                sc = scpool.tile([P, QB], F32, tag="sc")
                nc.vector.tensor_mul(sc[:], pair[et][:], recip[:])
                scs.append(sc)
            for et in range(DC):
                osl = slice(et * S + nb * QB, et * S + (nb + 1) * QB)
                ae = nc.gpsimd if et == 1 else nc.vector
                ae.tensor_add(out_acc[:, osl], out_acc[:, osl], scs[et][:])
                eng = nc.sync if et == 0 else nc.gpsimd
                eng.dma_start(
                    outd[et * P:(et + 1) * P, nb * QB:(nb + 1) * QB],
                    out_acc[:, osl])



_progs = {}


def _get_prog(repeat=1):
    if repeat not in _progs:
        _progs[repeat] = build_program(repeat)
    return _progs[repeat]


def _prepare_in_maps(k, v, q, Wk, bk, Wv, bv, Wq, bq, Wo, bo):
    scale = np.float32(1.0 / 16.0)  # 1/sqrt(D), exact power of two
    in_maps = []
    for core in range(2 * B):
        b, g = core // 2, core % 2
        hs = list(range(g * HPC, (g + 1) * HPC))
        wm = np.stack([
            (Wk[h].astype(np.float64)
             @ (Wq[h].astype(np.float64) * scale).T).astype(np.float32)
            for h in hs])
        w2 = np.stack([
            (Wv[h].astype(np.float64)
             @ Wo[h::H].astype(np.float64)).astype(np.float32)
            for h in hs])
        ku = np.stack([
            (k[b].astype(np.float64)
             @ (Wk[h].astype(np.float64) @ (bq[h].astype(np.float64) * scale))
             ).astype(np.float32).reshape(MT, P).T
            for h in hs])
        in_maps.append({
            "kt": np.ascontiguousarray(k[b].T),
            "vt": np.ascontiguousarray(v[b].T),
            "qt": np.ascontiguousarray(q[b].T),
            "wm": np.ascontiguousarray(wm),
            "w2": np.ascontiguousarray(w2),
            "ku": np.ascontiguousarray(ku),
        })
    return in_maps


def _bo_prime(bv, Wo, bo):
    acc = bo.astype(np.float64).copy()
    for h in range(H):
        acc += bv[h].astype(np.float64) @ Wo[h::H].astype(np.float64)
    return acc.astype(np.float32)


def _run_spmd(in_maps, repeat=1, **kwargs):
    nc = _get_prog(repeat)
    return run_bass_kernel_spmd(nc, in_maps, core_ids=list(range(2 * B)),
                                **kwargs)


def kernel(k, v, q, Wk, bk, Wv, bv, Wq, bq, Wo, bo):
    arrs = [np.asarray(x, dtype=np.float32)
            for x in (k, v, q, Wk, bk, Wv, bv, Wq, bq, Wo, bo)]
    k, v, q, Wk, bk, Wv, bv, Wq, bq, Wo, bo = arrs
    in_maps = _prepare_in_maps(k, v, q, Wk, bk, Wv, bv, Wq, bq, Wo, bo)
    rr = _run_spmd(in_maps)
    bop = _bo_prime(bv, Wo, bo)
    out = np.empty((B, S, D), np.float32)
    for b in range(B):
        out[b] = (rr.results[2 * b]["out"].T + rr.results[2 * b + 1]["out"].T
                  + bop)
    return out
